# revision 6
# baseline (speedup 1.0000x reference)
"""Causal self-attention TRN2 kernel (8 NeuronCores).

Problem: x[4,2048,1024] f32, w_qkv[3072,1024], w_proj[1024,1024]
  qkv = x @ w_qkv.T; per-head causal softmax(q k^T / sqrt(64)) v; out @ w_proj.T

Sharding: 8 cores = (head-group hg in {0,1}) x (batch b in {0..3}).
  Core computes its 8 heads for its batch; partial y (contracted over its
  512 channels of w_proj input dim) is summed pairwise on host.

Per-core dataflow (all matmul inputs float32r = full-rate TF32-like):
  A) QKV: qkT [1024,2048] (q,k transposed: f on partitions) and
     V [2048, 8x65] (natural; col 65k+64 = ones for the softmax denominator)
  B) per (i-block 512, head): S^T tiles [j=128,i=512] via PE (K=64),
     exp via ACT (scale=1/8) psum->sbuf, causal mask on straddling tiles via
     gpsimd.affine_select, PV via PE with lhsT=[V|1] -> psum [65,512]
     (row 64 = denom), normalize via DVE recip + gpsimd partition_broadcast
     + DVE mul -> attnT [512, 2048] (c_local on partitions)
  C) proj: attnT.T @ w_projT -> psum -> DMA straight to DRAM
"""

import numpy as np

import concourse.bacc as bacc
import concourse.mybir as mybir
import concourse.tile as tile
from concourse.bass_utils import run_bass_kernel_spmd

F32 = mybir.dt.float32
F32R = mybir.dt.float32r
EXP = mybir.ActivationFunctionType.Exp

B, T, C = 4, 2048, 1024
NH, HD = 16, 64
HPC = 8                      # heads per core
FH = HPC * HD                # 512: per-core q/k/v feature width
NCORES = 8

_CACHE = {}


def build_nc():
    nc = bacc.Bacc()
    xT_d = nc.dram_tensor("xT", [C, T], F32R, kind="ExternalInput")
    wqkvT_d = nc.dram_tensor("wqkvT", [C, 3 * FH], F32R, kind="ExternalInput")
    wprojT_d = nc.dram_tensor("wprojT", [FH, C], F32R, kind="ExternalInput")
    y_d = nc.dram_tensor("y", [T, C], F32, kind="ExternalOutput")

    NKT = C // 128           # 8 c-tiles (contraction for qkv)
    NTT = T // 128           # 16 t-tiles
    NTC = T // 512           # 4 t-chunks / i-blocks

    with tile.TileContext(nc) as tc:
        with (
            # ---------------- persistent pools (whole kernel) --------------
            tc.tile_pool(name="qkt", bufs=1) as qkt_pool,
            tc.tile_pool(name="vp", bufs=1) as v_pool,
            tc.tile_pool(name="wproj", bufs=1) as wproj_pool,
        ):
            qkT = [qkt_pool.tile([128, T], F32R, tag=f"qkt{i}", name=f"qkt{i}")
                   for i in range(8)]
            v_sb = [v_pool.tile([128, HPC * 65], F32R, tag=f"v{i}", name=f"v{i}")
                    for i in range(NTT)]
            wprojT = [wproj_pool.tile([128, C], F32R, tag=f"wp{i}", name=f"wp{i}")
                      for i in range(4)]
            for g in range(4):
                nc.sync.dma_start(out=wprojT[g][:],
                                  in_=wprojT_d[g * 128:(g + 1) * 128, :])

            # ---------------- phase A: QKV projections ---------------------
            with (
                tc.tile_pool(name="wq", bufs=1) as w_pool,
                tc.tile_pool(name="xc", bufs=1) as x_pool,
                tc.tile_pool(name="psA", bufs=1, space="PSUM") as psA,
            ):
                wq = [w_pool.tile([128, 3 * FH], F32R, tag=f"wq{k}", name=f"wq{k}")
                      for k in range(NKT)]
                for k in range(NKT):
                    nc.sync.dma_start(out=wq[k][:],
                                      in_=wqkvT_d[k * 128:(k + 1) * 128, :])

                for tcb in range(NTC):       # t-chunk of 512
                    xc = [x_pool.tile([128, 512], F32R,
                                      tag=f"xc{k}", bufs=2 if k < 4 else 1,
                                      name=f"xc{tcb}_{k}")
                          for k in range(NKT)]
                    for k in range(NKT):
                        nc.sync.dma_start(
                            out=xc[k][:],
                            in_=xT_d[k * 128:(k + 1) * 128,
                                     tcb * 512:(tcb + 1) * 512])
                    # q,k transposed: out[f-tile 128, t 512]
                    for fi in range(8):      # 0-3 q rows, 4-7 k rows
                        fcol = fi * 128      # within [q|k] = first 1024 cols of wq
                        ps = psA.tile([128, 512], F32, tag="psA",
                                      name=f"psqk{tcb}_{fi}")
                        for k in range(NKT):
                            nc.tensor.matmul(ps[:],
                                             wq[k][:, fcol:fcol + 128],
                                             xc[k][:],
                                             start=(k == 0), stop=(k == NKT - 1))
                        nc.vector.tensor_copy(
                            out=qkT[fi][:, tcb * 512:(tcb + 1) * 512], in_=ps[:])
                    # v natural: out[t-tile 128, f_v 512]
                    for tl in range(4):
                        ti = tcb * 4 + tl
                        ps = psA.tile([128, 512], F32, tag="psA",
                                      name=f"psv{ti}")
                        for k in range(NKT):
                            nc.tensor.matmul(ps[:],
                                             xc[k][:, tl * 128:(tl + 1) * 128],
                                             wq[k][:, 2 * FH:3 * FH],
                                             start=(k == 0), stop=(k == NKT - 1))
                        vt = v_sb[ti]
                        vv = vt[:].rearrange("p (h x) -> p h x", h=HPC)
                        nc.vector.memset(vt[:].bitcast(F32), 1.0)
                        nc.vector.tensor_copy(
                            out=vv[:, :, 0:64],
                            in_=ps[:].rearrange("p (h x) -> p h x", h=HPC))

            # ---------------- phase B + C: attention + projection ----------
            with (
                tc.tile_pool(name="attnt", bufs=1) as attnt_pool,
                tc.tile_pool(name="pt", bufs=1) as pt_pool,
                tc.tile_pool(name="stage", bufs=1) as stage_pool,
                tc.tile_pool(name="nrm", bufs=1) as nrm_pool,
                tc.tile_pool(name="psS", bufs=1, space="PSUM") as psS,
                tc.tile_pool(name="psPV", bufs=1, space="PSUM") as psPV,
                tc.tile_pool(name="psO", bufs=1, space="PSUM") as psO,
            ):
                attnT = [attnt_pool.tile([128, T], F32R, tag=f"at{g}",
                                         name=f"at{g}") for g in range(4)]
                ost_pool = stage_pool
                for bi in range(NTC):
                    for h in range(HPC):
                        qt = qkT[h // 2]
                        kt = qkT[4 + h // 2]
                        off = (h % 2) * 64
                        npairs = 2 * bi + 2
                        njt = 4 * bi + 4
                        pts = []
                        for pj in range(npairs):
                            sps = psS.tile([128, 1024], F32, tag="sps", bufs=2,
                                           name=f"sps{bi}_{h}_{pj}")
                            for b2 in range(2):
                                jj = 2 * pj + b2
                                nc.tensor.matmul(
                                    sps[:, b2 * 512:(b2 + 1) * 512],
                                    kt[off:off + 64, jj * 128:(jj + 1) * 128],
                                    qt[off:off + 64, bi * 512:(bi + 1) * 512],
                                    start=True, stop=True)
                            pt = pt_pool.tile([128, 1024], F32R, tag="pt",
                                              bufs=4, name=f"pt{bi}_{h}_{pj}")
                            if pj < 2 * bi:
                                nc.scalar.activation(pt[:], sps[:], EXP,
                                                     scale=0.125)
                            else:
                                st = stage_pool.tile([128, 1024], F32R,
                                                     tag="st", bufs=2,
                                                     name=f"st{bi}_{h}_{pj}")
                                nc.scalar.activation(st[:], sps[:], EXP,
                                                     scale=0.125)
                                r0 = 2 * pj - 4 * bi
                                nc.gpsimd.affine_select(
                                    out=pt[:].rearrange("p (b i) -> p b i", b=2),
                                    in_=st[:].rearrange("p (b i) -> p b i", b=2),
                                    compare_op=mybir.AluOpType.is_ge,
                                    fill=0.0,
                                    base=-128 * r0,
                                    pattern=[[-128, 2], [1, 512]],
                                    channel_multiplier=-1,
                                )
                            pts.append(pt)
                        pv = psPV.tile([65, 512], F32, tag="pv", bufs=2,
                                       name=f"pv{bi}_{h}")
                        for jj in range(njt):
                            nc.tensor.matmul(
                                pv[:],
                                v_sb[jj][:, h * 65:h * 65 + 65],
                                pts[jj // 2][:, (jj % 2) * 512:(jj % 2 + 1) * 512],
                                start=(jj == 0), stop=(jj == njt - 1))
                        den = nrm_pool.tile([1, 512], F32, tag="den", bufs=2,
                                            name=f"den{bi}_{h}")
                        nc.vector.tensor_copy(out=den[0:1, :], in_=pv[64:65, :])
                        rec = nrm_pool.tile([1, 512], F32, tag="rec", bufs=2,
                                            name=f"rec{bi}_{h}")
                        nc.vector.reciprocal(out=rec[0:1, :], in_=den[0:1, :])
                        bc = nrm_pool.tile([64, 512], F32, tag="bc", bufs=2,
                                           name=f"bc{bi}_{h}")
                        nc.gpsimd.partition_broadcast(bc[:, :], rec[0:1, :])
                        g = h // 2
                        off2 = (h % 2) * 64
                        nc.vector.tensor_mul(
                            out=attnT[g][off2:off2 + 64,
                                         bi * 512:(bi + 1) * 512],
                            in0=pv[0:64, :], in1=bc[:, :])
                    # ---- projection for the t-tiles of this i-block ----
                    for tl in range(4):
                        ti = bi * 4 + tl
                        for fc in range(2):
                            po = psO.tile([128, 512], F32, tag="po", bufs=2,
                                          name=f"po{ti}_{fc}")
                            for g in range(4):
                                nc.tensor.matmul(
                                    po[:],
                                    attnT[g][:, ti * 128:(ti + 1) * 128],
                                    wprojT[g][:, fc * 512:(fc + 1) * 512],
                                    start=(g == 0), stop=(g == 3))
                            ot = ost_pool.tile([128, 512], F32, tag="ot",
                                               bufs=3, name=f"ot{ti}_{fc}")
                            nc.vector.tensor_copy(out=ot[:], in_=po[:])
                            nc.sync.dma_start(
                                out=y_d[ti * 128:(ti + 1) * 128,
                                        fc * 512:(fc + 1) * 512],
                                in_=ot[:])
    nc.compile()
    return nc


def _get_nc():
    if "nc" not in _CACHE:
        _CACHE["nc"] = build_nc()
    return _CACHE["nc"]


def kernel(x, w_qkv, w_proj, _trace=False):
    x = np.asarray(x, dtype=np.float32)
    w_qkv = np.asarray(w_qkv, dtype=np.float32)
    w_proj = np.asarray(w_proj, dtype=np.float32)

    nc = _get_nc()
    in_maps = []
    for c in range(NCORES):
        hg, b = c // 4, c % 4
        xT = np.ascontiguousarray(x[b].T)                       # [1024, 2048]
        rows = []
        for sec in range(3):                                     # q, k, v
            rows.append(w_qkv[sec * C + hg * FH: sec * C + (hg + 1) * FH])
        wqkvT = np.ascontiguousarray(np.concatenate(rows, 0).T)  # [1024, 1536]
        wprojT = np.ascontiguousarray(w_proj[:, hg * FH:(hg + 1) * FH].T)
        in_maps.append({"xT": xT, "wqkvT": wqkvT, "wprojT": wprojT})

    res = run_bass_kernel_spmd(nc, in_maps, list(range(NCORES)), trace=_trace)
    if _trace:
        _CACHE["exec_time_ns"] = res.exec_time_ns

    y = np.empty((B, T, C), dtype=np.float32)
    for b in range(B):
        y[b] = res.results[b]["y"] + res.results[4 + b]["y"]
    return y


# revision 7
# speedup vs baseline: 1.2819x; 1.2819x over previous
"""Causal self-attention TRN2 kernel (8 NeuronCores).

Problem: x[4,2048,1024] f32, w_qkv[3072,1024], w_proj[1024,1024]
  qkv = x @ w_qkv.T; per-head causal softmax(q k^T / sqrt(64)) v; out @ w_proj.T

Sharding: 8 cores = (head-group hg in {0,1}) x (batch b in {0..3}).
  Core computes its 8 heads for its batch; partial y (contracted over its
  512 channels of w_proj input dim) is summed pairwise on host.

Per-core dataflow (all matmul inputs float32r = full-rate TF32-like):
  A) QKV: qkT [1024,2048] (q,k transposed: f on partitions) and
     V [2048, 8x65] (natural; col 65k+64 = ones for the softmax denominator)
  B) per (i-block 512, head): S^T tiles [j=128,i=512] via PE (K=64),
     exp via ACT (scale=1/8) psum->sbuf, causal mask on straddling tiles via
     gpsimd.affine_select, PV via PE with lhsT=[V|1] -> psum [65,512]
     (row 64 = denom), normalize via DVE recip + gpsimd partition_broadcast
     + DVE mul -> attnT [512, 2048] (c_local on partitions)
  C) proj: attnT.T @ w_projT -> psum -> DMA straight to DRAM
"""

import numpy as np

import concourse.bacc as bacc
import concourse.mybir as mybir
import concourse.tile as tile
from concourse.bass_utils import run_bass_kernel_spmd

F32 = mybir.dt.float32
F32R = mybir.dt.float32r
EXP = mybir.ActivationFunctionType.Exp

B, T, C = 4, 2048, 1024
NH, HD = 16, 64
HPC = 8                      # heads per core
FH = HPC * HD                # 512: per-core q/k/v feature width
NCORES = 8

_CACHE = {}


def build_nc():
    nc = bacc.Bacc()
    xT_d = nc.dram_tensor("xT", [C, T], F32R, kind="ExternalInput")
    wqkvT_d = nc.dram_tensor("wqkvT", [C, 3 * FH], F32R, kind="ExternalInput")
    wprojT_d = nc.dram_tensor("wprojT", [FH, C], F32R, kind="ExternalInput")
    y_d = nc.dram_tensor("y", [T, C], F32, kind="ExternalOutput")

    NKT = C // 128           # 8 c-tiles (contraction for qkv)
    NTT = T // 128           # 16 t-tiles
    NTC = T // 512           # 4 t-chunks / i-blocks

    with tile.TileContext(nc) as tc:
        with (
            # ---------------- persistent pools (whole kernel) --------------
            tc.tile_pool(name="qkt", bufs=1) as qkt_pool,
            tc.tile_pool(name="vp", bufs=1) as v_pool,
            tc.tile_pool(name="wproj", bufs=1) as wproj_pool,
        ):
            qkT = [qkt_pool.tile([128, T], F32R, tag=f"qkt{i}", name=f"qkt{i}")
                   for i in range(8)]
            v_sb = [v_pool.tile([128, HPC * 65], F32R, tag=f"v{i}", name=f"v{i}")
                    for i in range(NTT)]
            wprojT = [wproj_pool.tile([128, C], F32R, tag=f"wp{i}", name=f"wp{i}")
                      for i in range(4)]
            for g in range(4):
                nc.sync.dma_start(out=wprojT[g][:],
                                  in_=wprojT_d[g * 128:(g + 1) * 128, :])

            # ---------------- phase A: QKV projections ---------------------
            with (
                tc.tile_pool(name="wq", bufs=1) as w_pool,
                tc.tile_pool(name="xc", bufs=1) as x_pool,
                tc.tile_pool(name="psA", bufs=1, space="PSUM") as psA,
            ):
                wq = [w_pool.tile([128, 3 * FH], F32R, tag=f"wq{k}", name=f"wq{k}")
                      for k in range(NKT)]
                for k in range(NKT):
                    nc.sync.dma_start(out=wq[k][:],
                                      in_=wqkvT_d[k * 128:(k + 1) * 128, :])

                for tcb in range(NTC):       # t-chunk of 512
                    xc = [x_pool.tile([128, 512], F32R,
                                      tag=f"xc{k}", bufs=2 if k < 4 else 1,
                                      name=f"xc{tcb}_{k}")
                          for k in range(NKT)]
                    for k in range(NKT):
                        nc.sync.dma_start(
                            out=xc[k][:],
                            in_=xT_d[k * 128:(k + 1) * 128,
                                     tcb * 512:(tcb + 1) * 512])
                    # q,k transposed: out[f-tile 128, t 512]
                    for fi in range(8):      # 0-3 q rows, 4-7 k rows
                        fcol = fi * 128      # within [q|k] = first 1024 cols of wq
                        ps = psA.tile([128, 512], F32, tag="psA",
                                      name=f"psqk{tcb}_{fi}")
                        for k in range(NKT):
                            nc.tensor.matmul(ps[:],
                                             wq[k][:, fcol:fcol + 128],
                                             xc[k][:],
                                             start=(k == 0), stop=(k == NKT - 1))
                        nc.vector.tensor_copy(
                            out=qkT[fi][:, tcb * 512:(tcb + 1) * 512], in_=ps[:])
                    # v natural: out[t-tile 128, f_v 512]
                    for tl in range(4):
                        ti = tcb * 4 + tl
                        ps = psA.tile([128, 512], F32, tag="psA",
                                      name=f"psv{ti}")
                        for k in range(NKT):
                            nc.tensor.matmul(ps[:],
                                             xc[k][:, tl * 128:(tl + 1) * 128],
                                             wq[k][:, 2 * FH:3 * FH],
                                             start=(k == 0), stop=(k == NKT - 1))
                        vt = v_sb[ti]
                        vv = vt[:].rearrange("p (h x) -> p h x", h=HPC)
                        nc.vector.memset(vt[:].bitcast(F32), 1.0)
                        nc.vector.tensor_copy(
                            out=vv[:, :, 0:64],
                            in_=ps[:].rearrange("p (h x) -> p h x", h=HPC))

            # ---------------- phase B + C: attention + projection ----------
            with (
                tc.tile_pool(name="attnt", bufs=1) as attnt_pool,
                tc.tile_pool(name="pt", bufs=1) as pt_pool,
                tc.tile_pool(name="stage", bufs=1) as stage_pool,
                tc.tile_pool(name="nrm", bufs=1) as nrm_pool,
                tc.tile_pool(name="psS", bufs=1, space="PSUM") as psS,
                tc.tile_pool(name="psPV", bufs=1, space="PSUM") as psPV,
                tc.tile_pool(name="psO", bufs=1, space="PSUM") as psO,
            ):
                attnT = [attnt_pool.tile([128, T], F32R, tag=f"at{g}",
                                         name=f"at{g}") for g in range(4)]
                ost_pool = stage_pool
                for bi in range(NTC):
                    njt = 4 * bi + 4
                    for hp in range(4):          # head pair (2hp, 2hp+1)
                        qt = qkT[hp]
                        kt = qkT[4 + hp]
                        pts = []
                        for jj in range(njt):
                            sps = psS.tile([128, 1024], F32, tag="sps", bufs=2,
                                           name=f"sps{bi}_{hp}_{jj}")
                            # even head on PE rows 0-63, odd head on rows
                            # 64-127 -> the two MMs run concurrently
                            for par in range(2):
                                off = par * 64
                                nc.tensor.matmul(
                                    sps[:, par * 512:(par + 1) * 512],
                                    kt[off:off + 64, jj * 128:(jj + 1) * 128],
                                    qt[off:off + 64, bi * 512:(bi + 1) * 512],
                                    start=True, stop=True)
                            pt = pt_pool.tile([128, 1024], F32R, tag="pt",
                                              bufs=5, name=f"pt{bi}_{hp}_{jj}")
                            if jj < 4 * bi:
                                nc.scalar.activation(pt[:], sps[:], EXP,
                                                     scale=0.125)
                            else:
                                st = stage_pool.tile([128, 1024], F32R,
                                                     tag="st", bufs=2,
                                                     name=f"st{bi}_{hp}_{jj}")
                                nc.scalar.activation(st[:], sps[:], EXP,
                                                     scale=0.125)
                                r0 = jj - 4 * bi
                                nc.gpsimd.affine_select(
                                    out=pt[:].rearrange("p (b i) -> p b i", b=2),
                                    in_=st[:].rearrange("p (b i) -> p b i", b=2),
                                    compare_op=mybir.AluOpType.is_ge,
                                    fill=0.0,
                                    base=-128 * r0,
                                    pattern=[[0, 2], [1, 512]],
                                    channel_multiplier=-1,
                                )
                            pts.append(pt)
                        pvs = [psPV.tile([65, 512], F32, tag="pv", bufs=2,
                                         name=f"pv{bi}_{hp}_{par}")
                               for par in range(2)]
                        for jj in range(njt):
                            for par in range(2):
                                h = 2 * hp + par
                                nc.tensor.matmul(
                                    pvs[par][:],
                                    v_sb[jj][:, h * 65:h * 65 + 65],
                                    pts[jj][:, par * 512:(par + 1) * 512],
                                    start=(jj == 0), stop=(jj == njt - 1))
                        for par in range(2):
                            pv = pvs[par]
                            den = nrm_pool.tile([1, 512], F32, tag="den",
                                                bufs=2, name=f"den{bi}_{hp}_{par}")
                            nc.vector.tensor_copy(out=den[0:1, :],
                                                  in_=pv[64:65, :])
                            rec = nrm_pool.tile([1, 512], F32, tag="rec",
                                                bufs=2, name=f"rec{bi}_{hp}_{par}")
                            nc.vector.reciprocal_approx_fast(
                                out=rec[0:1, :], in_=den[0:1, :])
                            bc = nrm_pool.tile([64, 512], F32, tag="bc",
                                               bufs=2, name=f"bc{bi}_{hp}_{par}")
                            nc.gpsimd.partition_broadcast(bc[:, :], rec[0:1, :])
                            nc.vector.tensor_mul(
                                out=attnT[hp][par * 64:par * 64 + 64,
                                              bi * 512:(bi + 1) * 512],
                                in0=pv[0:64, :], in1=bc[:, :])
                    # ---- projection for the t-tiles of this i-block ----
                    for tl in range(4):
                        ti = bi * 4 + tl
                        for fc in range(2):
                            po = psO.tile([128, 512], F32, tag="po", bufs=2,
                                          name=f"po{ti}_{fc}")
                            for g in range(4):
                                nc.tensor.matmul(
                                    po[:],
                                    attnT[g][:, ti * 128:(ti + 1) * 128],
                                    wprojT[g][:, fc * 512:(fc + 1) * 512],
                                    start=(g == 0), stop=(g == 3))
                            ot = ost_pool.tile([128, 512], F32, tag="ot",
                                               bufs=3, name=f"ot{ti}_{fc}")
                            nc.vector.tensor_copy(out=ot[:], in_=po[:])
                            nc.sync.dma_start(
                                out=y_d[ti * 128:(ti + 1) * 128,
                                        fc * 512:(fc + 1) * 512],
                                in_=ot[:])
    nc.compile()
    return nc


def _get_nc():
    if "nc" not in _CACHE:
        _CACHE["nc"] = build_nc()
    return _CACHE["nc"]


def kernel(x, w_qkv, w_proj, _trace=False):
    x = np.asarray(x, dtype=np.float32)
    w_qkv = np.asarray(w_qkv, dtype=np.float32)
    w_proj = np.asarray(w_proj, dtype=np.float32)

    nc = _get_nc()
    in_maps = []
    for c in range(NCORES):
        hg, b = c // 4, c % 4
        xT = np.ascontiguousarray(x[b].T)                       # [1024, 2048]
        rows = []
        for sec in range(3):                                     # q, k, v
            rows.append(w_qkv[sec * C + hg * FH: sec * C + (hg + 1) * FH])
        wqkvT = np.ascontiguousarray(np.concatenate(rows, 0).T)  # [1024, 1536]
        wprojT = np.ascontiguousarray(w_proj[:, hg * FH:(hg + 1) * FH].T)
        in_maps.append({"xT": xT, "wqkvT": wqkvT, "wprojT": wprojT})

    res = run_bass_kernel_spmd(nc, in_maps, list(range(NCORES)), trace=_trace)
    if _trace:
        _CACHE["exec_time_ns"] = res.exec_time_ns

    y = np.empty((B, T, C), dtype=np.float32)
    for b in range(B):
        y[b] = res.results[b]["y"] + res.results[4 + b]["y"]
    return y


# revision 9
# speedup vs baseline: 1.3237x; 1.0326x over previous
"""Causal self-attention TRN2 kernel (8 NeuronCores).

Problem: x[4,2048,1024] f32, w_qkv[3072,1024], w_proj[1024,1024]
  qkv = x @ w_qkv.T; per-head causal softmax(q k^T / sqrt(64)) v; out @ w_proj.T

Sharding: 8 cores = (head-group hg in {0,1}) x (batch b in {0..3}).
  Core computes its 8 heads for its batch; partial y (contracted over its
  512 channels of w_proj input dim) is summed pairwise on host.

Per-core dataflow (all matmul inputs float32r = full-rate TF32-like):
  A) QKV: qkT [1024,2048] (q,k transposed: f on partitions) and
     V [2048, 8x65] (natural; col 65k+64 = ones for the softmax denominator)
  B) per (i-block 512, head): S^T tiles [j=128,i=512] via PE (K=64),
     exp via ACT (scale=1/8) psum->sbuf, causal mask on straddling tiles via
     gpsimd.affine_select, PV via PE with lhsT=[V|1] -> psum [65,512]
     (row 64 = denom), normalize via DVE recip + gpsimd partition_broadcast
     + DVE mul -> attnT [512, 2048] (c_local on partitions)
  C) proj: attnT.T @ w_projT -> psum -> DMA straight to DRAM
"""

import numpy as np

import concourse.bacc as bacc
import concourse.mybir as mybir
import concourse.tile as tile
from concourse.bass_utils import run_bass_kernel_spmd

F32 = mybir.dt.float32
F32R = mybir.dt.float32r
EXP = mybir.ActivationFunctionType.Exp

B, T, C = 4, 2048, 1024
NH, HD = 16, 64
HPC = 8                      # heads per core
FH = HPC * HD                # 512: per-core q/k/v feature width
NCORES = 8

_CACHE = {}


def build_nc():
    nc = bacc.Bacc()
    xT_d = nc.dram_tensor("xT", [C, T], F32R, kind="ExternalInput")
    wqkvT_d = nc.dram_tensor("wqkvT", [C, 3 * FH], F32R, kind="ExternalInput")
    wprojT_d = nc.dram_tensor("wprojT", [FH, C], F32R, kind="ExternalInput")
    y_d = nc.dram_tensor("y", [T, C], F32, kind="ExternalOutput")

    NKT = C // 128           # 8 c-tiles (contraction for qkv)
    NTT = T // 128           # 16 t-tiles
    NTC = T // 512           # 4 t-chunks / i-blocks

    with tile.TileContext(nc) as tc:
        with (
            # ---------------- persistent pools (whole kernel) --------------
            tc.tile_pool(name="qkt", bufs=1) as qkt_pool,
            tc.tile_pool(name="vp", bufs=1) as v_pool,
            tc.tile_pool(name="wproj", bufs=1) as wproj_pool,
        ):
            qkT = [qkt_pool.tile([128, T], F32R, tag=f"qkt{i}", name=f"qkt{i}")
                   for i in range(8)]
            v_sb = [v_pool.tile([128, HPC * 65], F32R, tag=f"v{i}", name=f"v{i}")
                    for i in range(NTT)]
            wprojT = [wproj_pool.tile([128, C], F32R, tag=f"wp{i}", name=f"wp{i}")
                      for i in range(4)]
            for g in range(4):
                nc.sync.dma_start(out=wprojT[g][:],
                                  in_=wprojT_d[g * 128:(g + 1) * 128, :])

            # ---------------- phase A: QKV projections ---------------------
            with (
                tc.tile_pool(name="wq", bufs=1) as w_pool,
                tc.tile_pool(name="xc", bufs=1) as x_pool,
                tc.tile_pool(name="psA", bufs=1, space="PSUM") as psA,
            ):
                wq = [w_pool.tile([128, 3 * FH], F32R, tag=f"wq{k}", name=f"wq{k}")
                      for k in range(NKT)]
                for k in range(NKT):
                    nc.sync.dma_start(out=wq[k][:],
                                      in_=wqkvT_d[k * 128:(k + 1) * 128, :])

                for tcb in range(NTC):       # t-chunk of 512
                    xc = [x_pool.tile([128, 512], F32R,
                                      tag=f"xc{k}", bufs=2 if k < 4 else 1,
                                      name=f"xc{tcb}_{k}")
                          for k in range(NKT)]
                    for k in range(NKT):
                        nc.sync.dma_start(
                            out=xc[k][:],
                            in_=xT_d[k * 128:(k + 1) * 128,
                                     tcb * 512:(tcb + 1) * 512])
                    # q,k transposed: out[f-tile 128, t 512]
                    for fi in range(8):      # 0-3 q rows, 4-7 k rows
                        fcol = fi * 128      # within [q|k] = first 1024 cols of wq
                        ps = psA.tile([128, 512], F32, tag="psA",
                                      name=f"psqk{tcb}_{fi}")
                        for k in range(NKT):
                            nc.tensor.matmul(ps[:],
                                             wq[k][:, fcol:fcol + 128],
                                             xc[k][:],
                                             start=(k == 0), stop=(k == NKT - 1))
                        nc.vector.tensor_copy(
                            out=qkT[fi][:, tcb * 512:(tcb + 1) * 512], in_=ps[:])
                    # v natural: out[t-tile 128, f_v 512]
                    for tl in range(4):
                        ti = tcb * 4 + tl
                        ps = psA.tile([128, 512], F32, tag="psA",
                                      name=f"psv{ti}")
                        for k in range(NKT):
                            nc.tensor.matmul(ps[:],
                                             xc[k][:, tl * 128:(tl + 1) * 128],
                                             wq[k][:, 2 * FH:3 * FH],
                                             start=(k == 0), stop=(k == NKT - 1))
                        vt = v_sb[ti]
                        vv = vt[:].rearrange("p (h x) -> p h x", h=HPC)
                        nc.vector.memset(vt[:].bitcast(F32), 1.0)
                        nc.vector.tensor_copy(
                            out=vv[:, :, 0:64],
                            in_=ps[:].rearrange("p (h x) -> p h x", h=HPC))

            # ---------------- phase B + C: attention + projection ----------
            with (
                tc.tile_pool(name="attnt", bufs=1) as attnt_pool,
                tc.tile_pool(name="pt", bufs=1) as pt_pool,
                tc.tile_pool(name="stage", bufs=1) as stage_pool,
                tc.tile_pool(name="nrm", bufs=1) as nrm_pool,
                tc.tile_pool(name="psS", bufs=1, space="PSUM") as psS,
                tc.tile_pool(name="psPV", bufs=1, space="PSUM") as psPV,
                tc.tile_pool(name="psO", bufs=1, space="PSUM") as psO,
            ):
                attnT = [attnt_pool.tile([128, T], F32R, tag=f"at{g}",
                                         name=f"at{g}") for g in range(4)]
                ost_pool = stage_pool

                def emit_proj(ti, fc):
                    po = psO.tile([128, 512], F32, tag="po", bufs=2,
                                  name=f"po{ti}_{fc}")
                    for g in range(4):
                        nc.tensor.matmul(
                            po[:],
                            attnT[g][:, ti * 128:(ti + 1) * 128],
                            wprojT[g][:, fc * 512:(fc + 1) * 512],
                            start=(g == 0), stop=(g == 3))
                    ot = ost_pool.tile([128, 512], F32, tag="ot",
                                       bufs=3, name=f"ot{ti}_{fc}")
                    nc.vector.tensor_copy(out=ot[:], in_=po[:])
                    nc.sync.dma_start(
                        out=y_d[ti * 128:(ti + 1) * 128,
                                fc * 512:(fc + 1) * 512],
                        in_=ot[:])

                # (ti, fc) proj groups pending once their i-block's attnT done;
                # emitted lazily (2 per head-pair iteration of the NEXT
                # i-block) so PE never waits on the normalize chain.
                pending = []
                for bi in range(NTC):
                    njt = 4 * bi + 4
                    for hp in range(4):          # head pair (2hp, 2hp+1)
                        for _ in range(2):
                            if pending:
                                emit_proj(*pending.pop(0))
                        qt = qkT[hp]
                        kt = qkT[4 + hp]
                        pts = []
                        for jj in range(njt):
                            sps = psS.tile([128, 1024], F32, tag="sps", bufs=2,
                                           name=f"sps{bi}_{hp}_{jj}")
                            # even head on PE rows 0-63, odd head on rows
                            # 64-127 -> the two MMs run concurrently
                            for par in range(2):
                                off = par * 64
                                nc.tensor.matmul(
                                    sps[:, par * 512:(par + 1) * 512],
                                    kt[off:off + 64, jj * 128:(jj + 1) * 128],
                                    qt[off:off + 64, bi * 512:(bi + 1) * 512],
                                    start=True, stop=True)
                            pt = pt_pool.tile([128, 1024], F32R, tag="pt",
                                              bufs=5, name=f"pt{bi}_{hp}_{jj}")
                            if jj < 4 * bi:
                                nc.scalar.activation(pt[:], sps[:], EXP,
                                                     scale=0.125)
                            else:
                                st = stage_pool.tile([128, 1024], F32R,
                                                     tag="st", bufs=2,
                                                     name=f"st{bi}_{hp}_{jj}")
                                nc.scalar.activation(st[:], sps[:], EXP,
                                                     scale=0.125)
                                r0 = jj - 4 * bi
                                nc.gpsimd.affine_select(
                                    out=pt[:].rearrange("p (b i) -> p b i", b=2),
                                    in_=st[:].rearrange("p (b i) -> p b i", b=2),
                                    compare_op=mybir.AluOpType.is_ge,
                                    fill=0.0,
                                    base=-128 * r0,
                                    pattern=[[0, 2], [1, 512]],
                                    channel_multiplier=-1,
                                )
                            pts.append(pt)
                        pvs = [psPV.tile([65, 512], F32, tag="pv", bufs=2,
                                         name=f"pv{bi}_{hp}_{par}")
                               for par in range(2)]
                        for jj in range(njt):
                            for par in range(2):
                                h = 2 * hp + par
                                nc.tensor.matmul(
                                    pvs[par][:],
                                    v_sb[jj][:, h * 65:h * 65 + 65],
                                    pts[jj][:, par * 512:(par + 1) * 512],
                                    start=(jj == 0), stop=(jj == njt - 1))
                        for par in range(2):
                            pv = pvs[par]
                            den = nrm_pool.tile([1, 512], F32, tag="den",
                                                bufs=2, name=f"den{bi}_{hp}_{par}")
                            nc.vector.tensor_copy(out=den[0:1, :],
                                                  in_=pv[64:65, :])
                            rec = nrm_pool.tile([1, 512], F32, tag="rec",
                                                bufs=2, name=f"rec{bi}_{hp}_{par}")
                            nc.vector.reciprocal_approx_fast(
                                out=rec[0:1, :], in_=den[0:1, :])
                            bc = nrm_pool.tile([64, 512], F32, tag="bc",
                                               bufs=2, name=f"bc{bi}_{hp}_{par}")
                            nc.gpsimd.partition_broadcast(bc[:, :], rec[0:1, :])
                            nc.vector.tensor_mul(
                                out=attnT[hp][par * 64:par * 64 + 64,
                                              bi * 512:(bi + 1) * 512],
                                in0=pv[0:64, :], in1=bc[:, :])
                    # queue this i-block's projection groups
                    for tl in range(4):
                        for fc in range(2):
                            pending.append((bi * 4 + tl, fc))
                for pf in pending:
                    emit_proj(*pf)
    nc.compile()
    return nc


def _get_nc():
    if "nc" not in _CACHE:
        _CACHE["nc"] = build_nc()
    return _CACHE["nc"]


def kernel(x, w_qkv, w_proj, _trace=False):
    x = np.asarray(x, dtype=np.float32)
    w_qkv = np.asarray(w_qkv, dtype=np.float32)
    w_proj = np.asarray(w_proj, dtype=np.float32)

    nc = _get_nc()
    in_maps = []
    for c in range(NCORES):
        hg, b = c // 4, c % 4
        xT = np.ascontiguousarray(x[b].T)                       # [1024, 2048]
        rows = []
        for sec in range(3):                                     # q, k, v
            rows.append(w_qkv[sec * C + hg * FH: sec * C + (hg + 1) * FH])
        wqkvT = np.ascontiguousarray(np.concatenate(rows, 0).T)  # [1024, 1536]
        wprojT = np.ascontiguousarray(w_proj[:, hg * FH:(hg + 1) * FH].T)
        in_maps.append({"xT": xT, "wqkvT": wqkvT, "wprojT": wprojT})

    res = run_bass_kernel_spmd(nc, in_maps, list(range(NCORES)), trace=_trace)
    if _trace:
        _CACHE["exec_time_ns"] = res.exec_time_ns

    y = np.empty((B, T, C), dtype=np.float32)
    for b in range(B):
        y[b] = res.results[b]["y"] + res.results[4 + b]["y"]
    return y


# revision 10
# speedup vs baseline: 1.5496x; 1.1706x over previous
"""Causal self-attention TRN2 kernel (8 NeuronCores).

Problem: x[4,2048,1024] f32, w_qkv[3072,1024], w_proj[1024,1024]
  qkv = x @ w_qkv.T; per-head causal softmax(q k^T / sqrt(64)) v; out @ w_proj.T

Sharding: 8 cores = (head-group hg in {0,1}) x (batch b in {0..3}).
  Core computes its 8 heads for its batch; partial y (contracted over its
  512 channels of w_proj input dim) is summed pairwise on host.

Per-core dataflow (all matmul inputs float32r = full-rate TF32-like):
  A) QKV: qkT [1024,2048] (q,k transposed: f on partitions) and
     V [2048, 8x65] (natural; col 65k+64 = ones for the softmax denominator)
  B) per (i-block 512, head): S^T tiles [j=128,i=512] via PE (K=64),
     exp via ACT (scale=1/8) psum->sbuf, causal mask on straddling tiles via
     gpsimd.affine_select, PV via PE with lhsT=[V|1] -> psum [65,512]
     (row 64 = denom), normalize via DVE recip + gpsimd partition_broadcast
     + DVE mul -> attnT [512, 2048] (c_local on partitions)
  C) proj: attnT.T @ w_projT -> psum -> DMA straight to DRAM
"""

import numpy as np

import concourse.bacc as bacc
import concourse.mybir as mybir
import concourse.tile as tile
from concourse.bass_utils import run_bass_kernel_spmd

F32 = mybir.dt.float32
F32R = mybir.dt.float32r
EXP = mybir.ActivationFunctionType.Exp

B, T, C = 4, 2048, 1024
NH, HD = 16, 64
HPC = 8                      # heads per core
FH = HPC * HD                # 512: per-core q/k/v feature width
NCORES = 8

_CACHE = {}


def build_nc():
    nc = bacc.Bacc()
    xT_d = nc.dram_tensor("xT", [C, T], F32R, kind="ExternalInput")
    wqkvT_d = nc.dram_tensor("wqkvT", [C, 3 * FH], F32R, kind="ExternalInput")
    wprojT_d = nc.dram_tensor("wprojT", [FH, C], F32R, kind="ExternalInput")
    y_d = nc.dram_tensor("y", [T, C], F32, kind="ExternalOutput")

    NKT = C // 128           # 8 c-tiles (contraction for qkv)
    NTT = T // 128           # 16 t-tiles
    NTC = T // 512           # 4 t-chunks / i-blocks

    with tile.TileContext(nc) as tc:
        with (
            # ---------------- persistent pools (whole kernel) --------------
            tc.tile_pool(name="qkt", bufs=1) as qkt_pool,
            tc.tile_pool(name="vp", bufs=1) as v_pool,
            tc.tile_pool(name="wproj", bufs=1) as wproj_pool,
        ):
            qkT = [qkt_pool.tile([128, T], F32R, tag=f"qkt{i}", name=f"qkt{i}")
                   for i in range(8)]
            v_sb = [v_pool.tile([128, HPC * 65], F32R, tag=f"v{i}", name=f"v{i}")
                    for i in range(NTT)]
            wprojT = [wproj_pool.tile([128, C], F32R, tag=f"wp{i}", name=f"wp{i}")
                      for i in range(4)]
            for g in range(4):
                nc.sync.dma_start(out=wprojT[g][:],
                                  in_=wprojT_d[g * 128:(g + 1) * 128, :])

            # ---------------- phase A: QKV projections ---------------------
            with (
                tc.tile_pool(name="wq", bufs=1) as w_pool,
                tc.tile_pool(name="xc", bufs=1) as x_pool,
                tc.tile_pool(name="psA", bufs=1, space="PSUM") as psA,
            ):
                wq = [w_pool.tile([128, 3 * FH], F32R, tag=f"wq{k}", name=f"wq{k}")
                      for k in range(NKT)]
                for k in range(NKT):
                    nc.sync.dma_start(out=wq[k][:],
                                      in_=wqkvT_d[k * 128:(k + 1) * 128, :])

                for tcb in range(NTC):       # t-chunk of 512
                    xc = [x_pool.tile([128, 512], F32R,
                                      tag=f"xc{k}", bufs=2 if k < 4 else 1,
                                      name=f"xc{tcb}_{k}")
                          for k in range(NKT)]
                    for k in range(NKT):
                        nc.sync.dma_start(
                            out=xc[k][:],
                            in_=xT_d[k * 128:(k + 1) * 128,
                                     tcb * 512:(tcb + 1) * 512])
                    # q,k transposed: out[f-tile 128, t 512]
                    for fi in range(8):      # 0-3 q rows, 4-7 k rows
                        fcol = fi * 128      # within [q|k] = first 1024 cols of wq
                        ps = psA.tile([128, 512], F32, tag="psA", bufs=6,
                                      name=f"psqk{tcb}_{fi}")
                        for k in range(NKT):
                            nc.tensor.matmul(ps[:],
                                             wq[k][:, fcol:fcol + 128],
                                             xc[k][:],
                                             start=(k == 0), stop=(k == NKT - 1))
                        nc.vector.tensor_copy(
                            out=qkT[fi][:, tcb * 512:(tcb + 1) * 512], in_=ps[:])
                    # v natural: out[t-tile 128, f_v 512]
                    for tl in range(4):
                        ti = tcb * 4 + tl
                        ps = psA.tile([128, 512], F32, tag="psA", bufs=6,
                                      name=f"psv{ti}")
                        for k in range(NKT):
                            nc.tensor.matmul(ps[:],
                                             xc[k][:, tl * 128:(tl + 1) * 128],
                                             wq[k][:, 2 * FH:3 * FH],
                                             start=(k == 0), stop=(k == NKT - 1))
                        vt = v_sb[ti]
                        vv = vt[:].rearrange("p (h x) -> p h x", h=HPC)
                        nc.vector.memset(vt[:].bitcast(F32), 1.0)
                        nc.vector.tensor_copy(
                            out=vv[:, :, 0:64],
                            in_=ps[:].rearrange("p (h x) -> p h x", h=HPC))

            # ---------------- phase B + C: attention + projection ----------
            with (
                tc.tile_pool(name="attnt", bufs=1) as attnt_pool,
                tc.tile_pool(name="pt", bufs=1) as pt_pool,
                tc.tile_pool(name="stage", bufs=1) as stage_pool,
                tc.tile_pool(name="nrm", bufs=1) as nrm_pool,
                tc.tile_pool(name="psS", bufs=1, space="PSUM") as psS,
                tc.tile_pool(name="psPV", bufs=1, space="PSUM") as psPV,
                tc.tile_pool(name="psO", bufs=1, space="PSUM") as psO,
            ):
                attnT = [attnt_pool.tile([128, T], F32R, tag=f"at{g}",
                                         name=f"at{g}") for g in range(4)]
                ost_pool = stage_pool

                def emit_proj(ti, fc):
                    po = psO.tile([128, 512], F32, tag="po", bufs=2,
                                  name=f"po{ti}_{fc}")
                    for g in range(4):
                        nc.tensor.matmul(
                            po[:],
                            attnT[g][:, ti * 128:(ti + 1) * 128],
                            wprojT[g][:, fc * 512:(fc + 1) * 512],
                            start=(g == 0), stop=(g == 3))
                    ot = ost_pool.tile([128, 512], F32, tag="ot",
                                       bufs=3, name=f"ot{ti}_{fc}")
                    nc.vector.tensor_copy(out=ot[:], in_=po[:])
                    nc.sync.dma_start(
                        out=y_d[ti * 128:(ti + 1) * 128,
                                fc * 512:(fc + 1) * 512],
                        in_=ot[:])

                # (ti, fc) proj groups pending once their i-block's attnT done;
                # emitted lazily (2 per head-pair iteration of the NEXT
                # i-block) so PE never waits on the normalize chain.
                pending = []
                for bi in range(NTC):
                    njt = 4 * bi + 4
                    for hp in range(4):          # head pair (2hp, 2hp+1)
                        for _ in range(2):
                            if pending:
                                emit_proj(*pending.pop(0))
                        qt = qkT[hp]
                        kt = qkT[4 + hp]
                        pts = []
                        for jj in range(njt):
                            sps = psS.tile([128, 1024], F32, tag="sps", bufs=2,
                                           name=f"sps{bi}_{hp}_{jj}")
                            # even head on PE rows 0-63, odd head on rows
                            # 64-127 -> the two MMs run concurrently
                            for par in range(2):
                                off = par * 64
                                nc.tensor.matmul(
                                    sps[:, par * 512:(par + 1) * 512],
                                    kt[off:off + 64, jj * 128:(jj + 1) * 128],
                                    qt[off:off + 64, bi * 512:(bi + 1) * 512],
                                    start=True, stop=True)
                            pt = pt_pool.tile([128, 1024], F32R, tag="pt",
                                              bufs=5, name=f"pt{bi}_{hp}_{jj}")
                            if jj < 4 * bi:
                                nc.scalar.activation(pt[:], sps[:], EXP,
                                                     scale=0.125)
                            else:
                                st = stage_pool.tile([128, 1024], F32R,
                                                     tag="st", bufs=2,
                                                     name=f"st{bi}_{hp}_{jj}")
                                nc.scalar.activation(st[:], sps[:], EXP,
                                                     scale=0.125)
                                r0 = jj - 4 * bi
                                nc.gpsimd.affine_select(
                                    out=pt[:].rearrange("p (b i) -> p b i", b=2),
                                    in_=st[:].rearrange("p (b i) -> p b i", b=2),
                                    compare_op=mybir.AluOpType.is_ge,
                                    fill=0.0,
                                    base=-128 * r0,
                                    pattern=[[0, 2], [1, 512]],
                                    channel_multiplier=-1,
                                )
                            pts.append(pt)
                        pvs = [psPV.tile([65, 512], F32, tag="pv", bufs=2,
                                         name=f"pv{bi}_{hp}_{par}")
                               for par in range(2)]
                        for jj in range(njt):
                            for par in range(2):
                                h = 2 * hp + par
                                nc.tensor.matmul(
                                    pvs[par][:],
                                    v_sb[jj][:, h * 65:h * 65 + 65],
                                    pts[jj][:, par * 512:(par + 1) * 512],
                                    start=(jj == 0), stop=(jj == njt - 1))
                        for par in range(2):
                            pv = pvs[par]
                            den = nrm_pool.tile([1, 512], F32, tag="den",
                                                bufs=2, name=f"den{bi}_{hp}_{par}")
                            nc.vector.tensor_copy(out=den[0:1, :],
                                                  in_=pv[64:65, :])
                            rec = nrm_pool.tile([1, 512], F32, tag="rec",
                                                bufs=2, name=f"rec{bi}_{hp}_{par}")
                            nc.vector.reciprocal_approx_fast(
                                out=rec[0:1, :], in_=den[0:1, :])
                            bc = nrm_pool.tile([64, 512], F32, tag="bc",
                                               bufs=2, name=f"bc{bi}_{hp}_{par}")
                            nc.gpsimd.partition_broadcast(bc[:, :], rec[0:1, :])
                            nc.vector.tensor_mul(
                                out=attnT[hp][par * 64:par * 64 + 64,
                                              bi * 512:(bi + 1) * 512],
                                in0=pv[0:64, :], in1=bc[:, :])
                    # queue this i-block's projection groups
                    for tl in range(4):
                        for fc in range(2):
                            pending.append((bi * 4 + tl, fc))
                for pf in pending:
                    emit_proj(*pf)
    nc.compile()
    return nc


def _get_nc():
    if "nc" not in _CACHE:
        _CACHE["nc"] = build_nc()
    return _CACHE["nc"]


def kernel(x, w_qkv, w_proj, _trace=False):
    x = np.asarray(x, dtype=np.float32)
    w_qkv = np.asarray(w_qkv, dtype=np.float32)
    w_proj = np.asarray(w_proj, dtype=np.float32)

    nc = _get_nc()
    in_maps = []
    for c in range(NCORES):
        hg, b = c // 4, c % 4
        xT = np.ascontiguousarray(x[b].T)                       # [1024, 2048]
        rows = []
        for sec in range(3):                                     # q, k, v
            rows.append(w_qkv[sec * C + hg * FH: sec * C + (hg + 1) * FH])
        wqkvT = np.ascontiguousarray(np.concatenate(rows, 0).T)  # [1024, 1536]
        wprojT = np.ascontiguousarray(w_proj[:, hg * FH:(hg + 1) * FH].T)
        in_maps.append({"xT": xT, "wqkvT": wqkvT, "wprojT": wprojT})

    res = run_bass_kernel_spmd(nc, in_maps, list(range(NCORES)), trace=_trace)
    if _trace:
        _CACHE["exec_time_ns"] = res.exec_time_ns

    y = np.empty((B, T, C), dtype=np.float32)
    for b in range(B):
        y[b] = res.results[b]["y"] + res.results[4 + b]["y"]
    return y


# revision 11
# speedup vs baseline: 1.5557x; 1.0040x over previous
"""Causal self-attention TRN2 kernel (8 NeuronCores).

Problem: x[4,2048,1024] f32, w_qkv[3072,1024], w_proj[1024,1024]
  qkv = x @ w_qkv.T; per-head causal softmax(q k^T / sqrt(64)) v; out @ w_proj.T

Sharding: 8 cores = (head-group hg in {0,1}) x (batch b in {0..3}).
  Core computes its 8 heads for its batch; partial y (contracted over its
  512 channels of w_proj input dim) is summed pairwise on host.

Per-core dataflow (all matmul inputs float32r = full-rate TF32-like):
  A) QKV: qkT [1024,2048] (q,k transposed: f on partitions) and
     V [2048, 8x65] (natural; col 65k+64 = ones for the softmax denominator)
  B) per (i-block 512, head): S^T tiles [j=128,i=512] via PE (K=64),
     exp via ACT (scale=1/8) psum->sbuf, causal mask on straddling tiles via
     gpsimd.affine_select, PV via PE with lhsT=[V|1] -> psum [65,512]
     (row 64 = denom), normalize via DVE recip + gpsimd partition_broadcast
     + DVE mul -> attnT [512, 2048] (c_local on partitions)
  C) proj: attnT.T @ w_projT -> psum -> DMA straight to DRAM
"""

import numpy as np

import concourse.bacc as bacc
import concourse.mybir as mybir
import concourse.tile as tile
from concourse.bass_utils import run_bass_kernel_spmd

F32 = mybir.dt.float32
F32R = mybir.dt.float32r
EXP = mybir.ActivationFunctionType.Exp

B, T, C = 4, 2048, 1024
NH, HD = 16, 64
HPC = 8                      # heads per core
FH = HPC * HD                # 512: per-core q/k/v feature width
NCORES = 8

_CACHE = {}


def build_nc():
    nc = bacc.Bacc()
    xT_d = nc.dram_tensor("xT", [C, T], F32R, kind="ExternalInput")
    wqkvT_d = nc.dram_tensor("wqkvT", [C, 3 * FH], F32R, kind="ExternalInput")
    wprojT_d = nc.dram_tensor("wprojT", [FH, C], F32R, kind="ExternalInput")
    y_d = nc.dram_tensor("y", [T, C], F32, kind="ExternalOutput")

    NKT = C // 128           # 8 c-tiles (contraction for qkv)
    NTT = T // 128           # 16 t-tiles
    NTC = T // 512           # 4 t-chunks / i-blocks

    with tile.TileContext(nc) as tc:
        with (
            # ---------------- persistent pools (whole kernel) --------------
            tc.tile_pool(name="qkt", bufs=1) as qkt_pool,
            tc.tile_pool(name="vp", bufs=1) as v_pool,
            tc.tile_pool(name="wproj", bufs=1) as wproj_pool,
        ):
            qkT = [qkt_pool.tile([128, T], F32R, tag=f"qkt{i}", name=f"qkt{i}")
                   for i in range(8)]
            v_sb = [v_pool.tile([128, HPC * 65], F32R, tag=f"v{i}", name=f"v{i}")
                    for i in range(NTT)]
            wprojT = [wproj_pool.tile([128, C], F32R, tag=f"wp{i}", name=f"wp{i}")
                      for i in range(4)]
            for g in range(4):
                nc.sync.dma_start(out=wprojT[g][:],
                                  in_=wprojT_d[g * 128:(g + 1) * 128, :])

            # ---------------- phase A: QKV projections ---------------------
            with (
                tc.tile_pool(name="wq", bufs=1) as w_pool,
                tc.tile_pool(name="xc", bufs=1) as x_pool,
                tc.tile_pool(name="psA", bufs=1, space="PSUM") as psA,
            ):
                wq = [w_pool.tile([128, 3 * FH], F32R, tag=f"wq{k}", name=f"wq{k}")
                      for k in range(NKT)]
                for k in range(NKT):
                    nc.sync.dma_start(out=wq[k][:],
                                      in_=wqkvT_d[k * 128:(k + 1) * 128, :])

                for tcb in range(NTC):       # t-chunk of 512
                    xc = [x_pool.tile([128, 512], F32R,
                                      tag=f"xc{k}", bufs=2 if k < 4 else 1,
                                      name=f"xc{tcb}_{k}")
                          for k in range(NKT)]
                    for k in range(NKT):
                        nc.sync.dma_start(
                            out=xc[k][:],
                            in_=xT_d[k * 128:(k + 1) * 128,
                                     tcb * 512:(tcb + 1) * 512])
                    # q,k transposed: out[f-tile 128, t 512]
                    for fi in range(8):      # 0-3 q rows, 4-7 k rows
                        fcol = fi * 128      # within [q|k] = first 1024 cols of wq
                        ps = psA.tile([128, 512], F32, tag="psA", bufs=6,
                                      name=f"psqk{tcb}_{fi}")
                        for k in range(NKT):
                            nc.tensor.matmul(ps[:],
                                             wq[k][:, fcol:fcol + 128],
                                             xc[k][:],
                                             start=(k == 0), stop=(k == NKT - 1))
                        nc.vector.tensor_copy(
                            out=qkT[fi][:, tcb * 512:(tcb + 1) * 512], in_=ps[:])
                    # v natural: out[t-tile 128, f_v 512]
                    for tl in range(4):
                        ti = tcb * 4 + tl
                        ps = psA.tile([128, 512], F32, tag="psA", bufs=6,
                                      name=f"psv{ti}")
                        for k in range(NKT):
                            nc.tensor.matmul(ps[:],
                                             xc[k][:, tl * 128:(tl + 1) * 128],
                                             wq[k][:, 2 * FH:3 * FH],
                                             start=(k == 0), stop=(k == NKT - 1))
                        vt = v_sb[ti]
                        vv = vt[:].rearrange("p (h x) -> p h x", h=HPC)
                        nc.vector.memset(vt[:].bitcast(F32), 1.0)
                        nc.vector.tensor_copy(
                            out=vv[:, :, 0:64],
                            in_=ps[:].rearrange("p (h x) -> p h x", h=HPC))

            # ---------------- phase B + C: attention + projection ----------
            with (
                tc.tile_pool(name="attnt", bufs=1) as attnt_pool,
                tc.tile_pool(name="pt", bufs=1) as pt_pool,
                tc.tile_pool(name="stage", bufs=1) as stage_pool,
                tc.tile_pool(name="nrm", bufs=1) as nrm_pool,
                tc.tile_pool(name="psS", bufs=1, space="PSUM") as psS,
                tc.tile_pool(name="psPV", bufs=1, space="PSUM") as psPV,
                tc.tile_pool(name="psO", bufs=1, space="PSUM") as psO,
            ):
                attnT = [attnt_pool.tile([128, T], F32R, tag=f"at{g}",
                                         name=f"at{g}") for g in range(4)]
                ost_pool = stage_pool

                def emit_proj(ti, fc):
                    po = psO.tile([128, 512], F32, tag="po", bufs=2,
                                  name=f"po{ti}_{fc}")
                    for g in range(4):
                        nc.tensor.matmul(
                            po[:],
                            attnT[g][:, ti * 128:(ti + 1) * 128],
                            wprojT[g][:, fc * 512:(fc + 1) * 512],
                            start=(g == 0), stop=(g == 3))
                    ot = ost_pool.tile([128, 512], F32, tag="ot",
                                       bufs=3, name=f"ot{ti}_{fc}")
                    nc.vector.tensor_copy(out=ot[:], in_=po[:])
                    nc.sync.dma_start(
                        out=y_d[ti * 128:(ti + 1) * 128,
                                fc * 512:(fc + 1) * 512],
                        in_=ot[:])

                # (ti, fc) proj groups pending once their i-block's attnT done;
                # emitted lazily (2 per head-pair iteration of the NEXT
                # i-block) so PE never waits on the normalize chain.
                pending = []
                LAG = 4
                for bi in range(NTC):
                    njt = 4 * bi + 4
                    for hp in range(4):          # head pair (2hp, 2hp+1)
                        for _ in range(2):
                            if pending:
                                emit_proj(*pending.pop(0))
                        qt = qkT[hp]
                        kt = qkT[4 + hp]
                        pts = []
                        pvs = [psPV.tile([65, 512], F32, tag="pv", bufs=2,
                                         name=f"pv{bi}_{hp}_{par}")
                               for par in range(2)]

                        def emit_scores(jj, bi=bi, hp=hp, qt=qt, kt=kt,
                                        pts=pts):
                            sps = psS.tile([128, 1024], F32, tag="sps", bufs=2,
                                           name=f"sps{bi}_{hp}_{jj}")
                            # even head on PE rows 0-63, odd head on rows
                            # 64-127 -> the two MMs run concurrently
                            for par in range(2):
                                off = par * 64
                                nc.tensor.matmul(
                                    sps[:, par * 512:(par + 1) * 512],
                                    kt[off:off + 64, jj * 128:(jj + 1) * 128],
                                    qt[off:off + 64, bi * 512:(bi + 1) * 512],
                                    start=True, stop=True)
                            pt = pt_pool.tile([128, 1024], F32R, tag="pt",
                                              bufs=LAG + 2,
                                              name=f"pt{bi}_{hp}_{jj}")
                            if jj < 4 * bi:
                                nc.scalar.activation(pt[:], sps[:], EXP,
                                                     scale=0.125)
                            else:
                                st = stage_pool.tile([128, 1024], F32R,
                                                     tag="st", bufs=2,
                                                     name=f"st{bi}_{hp}_{jj}")
                                nc.scalar.activation(st[:], sps[:], EXP,
                                                     scale=0.125)
                                r0 = jj - 4 * bi
                                nc.gpsimd.affine_select(
                                    out=pt[:].rearrange("p (b i) -> p b i", b=2),
                                    in_=st[:].rearrange("p (b i) -> p b i", b=2),
                                    compare_op=mybir.AluOpType.is_ge,
                                    fill=0.0,
                                    base=-128 * r0,
                                    pattern=[[0, 2], [1, 512]],
                                    channel_multiplier=-1,
                                )
                            pts.append(pt)

                        def emit_pv(jj, bi=bi, hp=hp, pts=pts, pvs=pvs,
                                    njt=njt):
                            for par in range(2):
                                h = 2 * hp + par
                                nc.tensor.matmul(
                                    pvs[par][:],
                                    v_sb[jj][:, h * 65:h * 65 + 65],
                                    pts[jj][:, par * 512:(par + 1) * 512],
                                    start=(jj == 0), stop=(jj == njt - 1))

                        # software pipeline: PV lags scores by LAG j-tiles so
                        # the PE never blocks on a fresh exp
                        for jj in range(njt):
                            emit_scores(jj)
                            if jj >= LAG:
                                emit_pv(jj - LAG)
                        for jj in range(max(0, njt - LAG), njt):
                            emit_pv(jj)
                        for par in range(2):
                            pv = pvs[par]
                            den = nrm_pool.tile([1, 512], F32, tag="den",
                                                bufs=2, name=f"den{bi}_{hp}_{par}")
                            nc.vector.tensor_copy(out=den[0:1, :],
                                                  in_=pv[64:65, :])
                            rec = nrm_pool.tile([1, 512], F32, tag="rec",
                                                bufs=2, name=f"rec{bi}_{hp}_{par}")
                            nc.vector.reciprocal_approx_fast(
                                out=rec[0:1, :], in_=den[0:1, :])
                            bc = nrm_pool.tile([64, 512], F32, tag="bc",
                                               bufs=2, name=f"bc{bi}_{hp}_{par}")
                            nc.gpsimd.partition_broadcast(bc[:, :], rec[0:1, :])
                            nc.vector.tensor_mul(
                                out=attnT[hp][par * 64:par * 64 + 64,
                                              bi * 512:(bi + 1) * 512],
                                in0=pv[0:64, :], in1=bc[:, :])
                    # queue this i-block's projection groups
                    for tl in range(4):
                        for fc in range(2):
                            pending.append((bi * 4 + tl, fc))
                for pf in pending:
                    emit_proj(*pf)
    nc.compile()
    return nc


def _get_nc():
    if "nc" not in _CACHE:
        _CACHE["nc"] = build_nc()
    return _CACHE["nc"]


def kernel(x, w_qkv, w_proj, _trace=False):
    x = np.asarray(x, dtype=np.float32)
    w_qkv = np.asarray(w_qkv, dtype=np.float32)
    w_proj = np.asarray(w_proj, dtype=np.float32)

    nc = _get_nc()
    in_maps = []
    for c in range(NCORES):
        hg, b = c // 4, c % 4
        xT = np.ascontiguousarray(x[b].T)                       # [1024, 2048]
        rows = []
        for sec in range(3):                                     # q, k, v
            rows.append(w_qkv[sec * C + hg * FH: sec * C + (hg + 1) * FH])
        wqkvT = np.ascontiguousarray(np.concatenate(rows, 0).T)  # [1024, 1536]
        wprojT = np.ascontiguousarray(w_proj[:, hg * FH:(hg + 1) * FH].T)
        in_maps.append({"xT": xT, "wqkvT": wqkvT, "wprojT": wprojT})

    res = run_bass_kernel_spmd(nc, in_maps, list(range(NCORES)), trace=_trace)
    if _trace:
        _CACHE["exec_time_ns"] = res.exec_time_ns

    y = np.empty((B, T, C), dtype=np.float32)
    for b in range(B):
        y[b] = res.results[b]["y"] + res.results[4 + b]["y"]
    return y
